# revision 16
# baseline (speedup 1.0000x reference)
"""CorrelationLayer kernel for 8 TRN2 NeuronCores.

corr[b,0,i,j] = sum_c fmap1[b,c,i,j] * mean_{k,l} fmap2[b,c,k,l]

Sharding: data-parallel over B (B=8 -> one sample per core). Per core:
  fmap2 [256, 9216] streams through SBUF; per-channel sums reduced on DVE.
  fmap1 [256, 9216] loaded resident in SBUF.
  m2 [256,1] (scaled by 1/9216) used as stationary matmul weights; fmap1
  streams through the PE in [128, 512] tiles, PSUM-accumulated over the
  two 128-channel blocks -> out [1, 9216].
"""

import numpy as np

import concourse.bass as bass
import concourse.tile as tile
from concourse import bacc, mybir
from concourse.bass_utils import run_bass_kernel_spmd

B, C, H, W = 8, 256, 96, 96
HW = H * W            # 9216
P = 128
KB = C // P           # 2 channel blocks
F2_TILES = 4          # fmap2 stream tiles per channel block
F2T = HW // F2_TILES  # 2304
N_T = 512             # matmul moving free dim (one PSUM bank)
N_TILES = HW // N_T   # 18
# fmap1 chunk widths per channel block: big chunks early for DMA
# efficiency, small chunks at the end so the tail drains quickly.
F1_WIDTHS = [1536] * 5 + [512] * 3
assert sum(F1_WIDTHS) == HW
DT = mybir.dt.float32

_NC_CACHE = []


def _build(loop_reps=None, dma_only=False, dual_ring=False):
    """loop_reps: if set, wrap the body in a hardware For-loop running it
    that many times — used only for device-time measurement (the per-call
    dispatch overhead through the PJRT tunnel dwarfs the kernel itself, so
    single-execution wall time is useless; the slope over reps isn't).
    dma_only: emit just the input DMAs (device DMA-floor measurement).
    dual_ring: issue fmap1 input DMAs on the ACT HWDGE ring instead of SP."""
    nc = bacc.Bacc("TRN2", debug=False)
    f1 = nc.dram_tensor("fmap1", [C, HW], DT, kind="ExternalInput").ap()
    f2 = nc.dram_tensor("fmap2", [C, HW], DT, kind="ExternalInput").ap()
    out = nc.dram_tensor("out", [1, HW], DT, kind="ExternalOutput").ap()

    with tile.TileContext(nc) as tc:
        ctx_loop = tc.For_i(0, loop_reps, 1) if loop_reps else None
        if ctx_loop is not None:
            ctx_loop.__enter__()
        with (
            tc.tile_pool(name="f2p", bufs=1) as f2p,
            tc.tile_pool(name="f1p", bufs=1) as f1p,
            tc.tile_pool(name="stat", bufs=1) as statp,
            tc.tile_pool(name="outp", bufs=1) as outp,
            tc.tile_pool(name="psp", bufs=8, space="PSUM") as psp,
        ):
            # All input tiles are resident (unique tags, no slot reuse) so
            # every input DMA has zero sync waits — HW DMA descriptors
            # support at most one wait condition.

            # --- fmap2: stream + per-tile channel-sum reduce ---
            parts = [
                statp.tile([P, F2_TILES], DT, name=f"part{kb}", tag=f"part{kb}")
                for kb in range(KB)
            ]
            for kb in range(KB):
                for t in range(F2_TILES):
                    f2t = f2p.tile(
                        [P, F2T], DT, name=f"f2_{kb}_{t}", tag=f"f2_{kb}_{t}"
                    )
                    nc.sync.dma_start(
                        out=f2t[:],
                        in_=f2[kb * P:(kb + 1) * P, t * F2T:(t + 1) * F2T],
                    )
                    if not dma_only:
                        nc.vector.reduce_sum(
                            parts[kb][:, t:t + 1], f2t[:], axis=mybir.AxisListType.X
                        )

            # --- fmap1: resident chunk loads, kb-interleaved so both
            # channel blocks of the same columns land back-to-back in the
            # HWDGE ring (nt groups complete as early as possible) ---
            f1c = {}
            f1_eng = nc.scalar if dual_ring else nc.sync
            starts = [sum(F1_WIDTHS[:j]) for j in range(len(F1_WIDTHS))]
            for j, (s0, w) in enumerate(zip(starts, F1_WIDTHS)):
                for kb in range(KB):
                    t_ = f1p.tile([P, w], DT, name=f"f1_{kb}_{j}", tag=f"f1_{kb}_{j}")
                    f1_eng.dma_start(
                        out=t_[:],
                        in_=f1[kb * P:(kb + 1) * P, s0:s0 + w],
                    )
                    f1c[(kb, j)] = t_

            if not dma_only:
                # --- m2 = per-channel sums of fmap2 (1/HW folded into the
                # psum->sbuf copy so matmuls wait only on the DVE reduce) ---
                m2 = []
                for kb in range(KB):
                    raw = statp.tile([P, 1], DT, name=f"m2r{kb}", tag=f"m2r{kb}")
                    nc.vector.reduce_sum(
                        raw[:], parts[kb][:], axis=mybir.AxisListType.X
                    )
                    m2.append(raw)

                # --- matvec chase: per column group j, both kb matmuls,
                # scaled copy to SBUF, flush per group ---
                out_sb = outp.tile([1, HW], DT, name="out_sb", tag="out_sb")
                for j, (s0, w) in enumerate(zip(starts, F1_WIDTHS)):
                    for g in range(w // N_T):
                        local = g * N_T
                        col = s0 + local
                        ps = psp.tile([1, N_T], DT, name="ps", tag="ps")
                        for kb in range(KB):
                            nc.tensor.matmul(
                                ps[:],
                                m2[kb][:],
                                f1c[(kb, j)][:, local:local + N_T],
                                start=(kb == 0),
                                stop=(kb == KB - 1),
                            )
                        nc.scalar.mul(
                            out_sb[:, col:col + N_T], ps[:], 1.0 / HW
                        )
                    nc.sync.dma_start(
                        out=out[:, s0:s0 + w],
                        in_=out_sb[:, s0:s0 + w],
                    )
        if ctx_loop is not None:
            ctx_loop.__exit__(None, None, None)

    nc.compile()
    return nc


def kernel(fmap1: np.ndarray, fmap2: np.ndarray) -> np.ndarray:
    fmap1 = np.ascontiguousarray(np.asarray(fmap1, dtype=np.float32))
    fmap2 = np.ascontiguousarray(np.asarray(fmap2, dtype=np.float32))
    assert fmap1.shape == (B, C, H, W) and fmap2.shape == (B, C, H, W)

    if not _NC_CACHE:
        _NC_CACHE.append(_build())
    nc = _NC_CACHE[0]

    in_maps = [
        {
            "fmap1": fmap1[b].reshape(C, HW),
            "fmap2": fmap2[b].reshape(C, HW),
        }
        for b in range(B)
    ]
    res = run_bass_kernel_spmd(nc, in_maps, core_ids=list(range(B)))
    out = np.stack(
        [res.results[b]["out"].reshape(1, H, W) for b in range(B)], axis=0
    )
    return out.astype(np.float32)


# revision 27
# speedup vs baseline: 1.1176x; 1.1176x over previous
"""CorrelationLayer kernel for 8 TRN2 NeuronCores.

corr[b,0,i,j] = sum_c fmap1[b,c,i,j] * mean_{k,l} fmap2[b,c,k,l]

Sharding: data-parallel over B (B=8 -> one sample per core). Per core:
  fmap2 [256, 9216] streams through SBUF; per-channel sums reduced on DVE.
  fmap1 [256, 9216] loaded resident in SBUF.
  m2 [256,1] (scaled by 1/9216) used as stationary matmul weights; fmap1
  streams through the PE in [128, 512] tiles, PSUM-accumulated over the
  two 128-channel blocks -> out [1, 9216].
"""

import numpy as np

import concourse.bass as bass
import concourse.tile as tile
from concourse import bacc, mybir
from concourse.bass_utils import run_bass_kernel_spmd

B, C, H, W = 8, 256, 96, 96
HW = H * W            # 9216
P = 128
KB = C // P           # 2 channel blocks
F2_TILES = 4          # fmap2 stream tiles per channel block
F2T = HW // F2_TILES  # 2304
N_T = 512             # matmul moving free dim (one PSUM bank)
N_TILES = HW // N_T   # 18
# fmap1 chunk widths per channel block: big chunks early for DMA
# efficiency, small chunks at the end so the tail drains quickly.
F1_WIDTHS = [1536] * 5 + [512] * 3
assert sum(F1_WIDTHS) == HW
DT = mybir.dt.float32
BF16 = mybir.dt.bfloat16

_NC_CACHE = []


def _build(loop_reps=None, dma_only=False, dual_ring=False, use_bf16=True):
    """loop_reps: if set, wrap the body in a hardware For-loop running it
    that many times — used only for device-time measurement (the per-call
    dispatch overhead through the PJRT tunnel dwarfs the kernel itself, so
    single-execution wall time is useless; the slope over reps isn't).
    dma_only: emit just the input DMAs (device DMA-floor measurement).
    dual_ring: issue fmap1 input DMAs on the ACT HWDGE ring instead of SP.
    use_bf16: cast matmul operands to bf16 (fp32 matmul streams at 1/4
    rate on the PE — 853ns vs 213ns per [128,512] tile — and falls behind
    the DMA; accumulation stays fp32 in PSUM)."""
    nc = bacc.Bacc("TRN2", debug=False)
    f1 = nc.dram_tensor("fmap1", [C, HW], DT, kind="ExternalInput").ap()
    f2 = nc.dram_tensor("fmap2", [C, HW], DT, kind="ExternalInput").ap()
    out = nc.dram_tensor("out", [1, HW], DT, kind="ExternalOutput").ap()

    with tile.TileContext(nc) as tc:
        ctx_loop = tc.For_i(0, loop_reps, 1) if loop_reps else None
        if ctx_loop is not None:
            ctx_loop.__enter__()
        with (
            tc.tile_pool(name="f2p", bufs=1) as f2p,
            tc.tile_pool(name="f1p", bufs=1) as f1p,
            tc.tile_pool(name="stat", bufs=1) as statp,
            tc.tile_pool(name="outp", bufs=1) as outp,
            tc.tile_pool(name="psp", bufs=8, space="PSUM") as psp,
        ):
            # All input tiles are resident (unique tags, no slot reuse) so
            # every input DMA has zero sync waits — HW DMA descriptors
            # support at most one wait condition.

            # --- fmap2: stream + per-tile channel-sum reduce ---
            parts = [
                statp.tile([P, F2_TILES], DT, name=f"part{kb}", tag=f"part{kb}")
                for kb in range(KB)
            ]
            for kb in range(KB):
                for t in range(F2_TILES):
                    f2t = f2p.tile(
                        [P, F2T], DT, name=f"f2_{kb}_{t}", tag=f"f2_{kb}_{t}"
                    )
                    nc.sync.dma_start(
                        out=f2t[:],
                        in_=f2[kb * P:(kb + 1) * P, t * F2T:(t + 1) * F2T],
                    )
                    if not dma_only:
                        nc.vector.reduce_sum(
                            parts[kb][:, t:t + 1], f2t[:], axis=mybir.AxisListType.X
                        )

            # --- fmap1: resident chunk loads, kb-interleaved so both
            # channel blocks of the same columns land back-to-back in the
            # HWDGE ring (nt groups complete as early as possible) ---
            f1c = {}
            f1_eng = nc.scalar if dual_ring else nc.sync
            starts = [sum(F1_WIDTHS[:j]) for j in range(len(F1_WIDTHS))]
            # fp32 arrivals rotate through a small pool when casting to
            # bf16 (the fp32 copy is dead after the cast); fully resident
            # otherwise (zero-wait DMAs keep the HWDGE ring flowing).
            stage_kw = (
                dict(tag="f1stage", bufs=8)
                if (use_bf16 and not dma_only)
                else {}
            )
            for j, (s0, w) in enumerate(zip(starts, F1_WIDTHS)):
                for kb in range(KB):
                    kw = stage_kw or dict(tag=f"f1_{kb}_{j}")
                    t_ = f1p.tile([P, w], DT, name=f"f1_{kb}_{j}", **kw)
                    f1_eng.dma_start(
                        out=t_[:],
                        in_=f1[kb * P:(kb + 1) * P, s0:s0 + w],
                    )
                    f1c[(kb, j)] = t_

            if not dma_only:
                # --- m2 = per-channel sums of fmap2 (1/HW folded into the
                # psum->sbuf copy so matmuls wait only on the DVE reduce) ---
                m2 = []
                for kb in range(KB):
                    raw = statp.tile([P, 1], DT, name=f"m2r{kb}", tag=f"m2r{kb}")
                    nc.vector.reduce_sum(
                        raw[:], parts[kb][:], axis=mybir.AxisListType.X
                    )
                    if use_bf16:
                        m2b = statp.tile([P, 1], BF16, name=f"m2b{kb}", tag=f"m2b{kb}")
                        nc.vector.tensor_copy(m2b[:], raw[:])
                        m2.append(m2b)
                    else:
                        m2.append(raw)

                # --- matvec chase as a feed-forward pipeline, one engine
                # per stage so no engine's program order ever waits on a
                # downstream stage:
                #   DVE: all bf16 casts (producer, waits only on DMAs)
                #   PE:  matmuls (waits DVE casts + PSUM slots)
                #   ACT: all psum-drain copies (waits PE)
                #   Pool SWDGE: out flushes (waits ACT; off the input ring)
                for j, (s0, w) in enumerate(zip(starts, F1_WIDTHS)):
                    if use_bf16:
                        rhs = {}
                        for kb in range(KB):
                            tb = f1p.tile(
                                [P, w], BF16, name=f"f1b_{kb}_{j}", tag=f"f1b_{kb}_{j}"
                            )
                            nc.vector.tensor_copy(tb[:], f1c[(kb, j)][:])
                            rhs[kb] = tb
                    else:
                        rhs = {kb: f1c[(kb, j)] for kb in range(KB)}
                    ob = outp.tile([1, w], DT, name=f"ob{j}", tag="ob", bufs=2)
                    for g in range(w // N_T):
                        local = g * N_T
                        ps = psp.tile([1, N_T], DT, name="ps", tag="ps", bufs=8)
                        for kb in range(KB):
                            nc.tensor.matmul(
                                ps[:],
                                m2[kb][:],
                                rhs[kb][:, local:local + N_T],
                                start=(kb == 0),
                                stop=(kb == KB - 1),
                            )
                        nc.scalar.mul(ob[:, local:local + N_T], ps[:], 1.0 / HW)
                    nc.gpsimd.dma_start(
                        out=out[:, s0:s0 + w],
                        in_=ob[:, :w],
                    )
        if ctx_loop is not None:
            ctx_loop.__exit__(None, None, None)

    nc.compile()
    return nc


def kernel(fmap1: np.ndarray, fmap2: np.ndarray) -> np.ndarray:
    fmap1 = np.ascontiguousarray(np.asarray(fmap1, dtype=np.float32))
    fmap2 = np.ascontiguousarray(np.asarray(fmap2, dtype=np.float32))
    assert fmap1.shape == (B, C, H, W) and fmap2.shape == (B, C, H, W)

    if not _NC_CACHE:
        _NC_CACHE.append(_build())
    nc = _NC_CACHE[0]

    in_maps = [
        {
            "fmap1": fmap1[b].reshape(C, HW),
            "fmap2": fmap2[b].reshape(C, HW),
        }
        for b in range(B)
    ]
    res = run_bass_kernel_spmd(nc, in_maps, core_ids=list(range(B)))
    out = np.stack(
        [res.results[b]["out"].reshape(1, H, W) for b in range(B)], axis=0
    )
    return out.astype(np.float32)
